# revision 12
# baseline (speedup 1.0000x reference)
"""Trainium2 Bass kernel for nn_Attention_78847009620267.

Reference computation (per batch b):
    t1 = fv[b] @ W_f + (b_f + hidden[b] @ W_h + b_h)     # [L, I]
    e  = t1 @ W_a + b_a                                  # [L, D]
    alpha = softmax(e, axis=L)                           # [L, D]
    z  = sum_l alpha * fv[b]                             # [D]
returns (z [B, D], alpha [B, L, D])

Strategy: data-parallel over batch B=64 across 8 NeuronCores (8 batches per
core), weights replicated.  The whole device kernel runs in the transposed
[D, L] domain so that softmax reductions and bias adds are native
per-partition operations; the host pre-transposes fv into [b, p, c, l] bf16
layout (d = c*128 + p) and post-transposes alpha back.  All matmuls run in
bf16 (fp32 accumulation in PSUM).  The device ships unnormalized exp(e),
its per-(b,d) sums S (free from the ScalarE activation accum_out), and the
raw weighted sums z_raw = sum_l exp*x; the host applies the 1/S
normalization during the gather (exactly the same arithmetic the device
would do, at full f32 precision).

No collectives are needed: each core's outputs depend only on its shard.
"""

import os
import sys

import numpy as np

sys.path.insert(0, "/opt/trn_rl_repo")

B, L, D, I = 64, 196, 2048, 512
NCORES = 8
BSH = B // NCORES          # batches per core = 8
DC = D // 128              # 16 d-chunks
IC = I // 128              # 4 i-chunks
NPAIR = BSH // 2           # process 2 batches per pass (N = 2*L = 392)

_CACHE = {}


def _build_nc():
    """Build the per-core Bass graph (same graph for all 8 cores)."""
    import concourse.bass as bass  # noqa: F401
    import concourse.mybir as mybir
    import concourse.tile as tile
    from concourse import bacc

    dt = mybir.dt
    AF = mybir.ActivationFunctionType
    ALU = mybir.AluOpType

    nc = bacc.Bacc("TRN2", target_bir_lowering=False, debug=False)

    # ---- I/O declarations (per-core shard layouts, see host code below) ----
    # xt[b, p, c, l] = bf16(fv[b, l, c*128 + p])
    xt = nc.dram_tensor("xt", [BSH, 128, DC, L], dt.bfloat16, kind="ExternalInput")
    # wf[p, k, i] = W_f[k*128 + p, i]  (bf16)
    wf = nc.dram_tensor("wf", [128, DC, I], dt.bfloat16, kind="ExternalInput")
    # wa[p, k, d] = W_a[k*128 + p, d]  (bf16)
    wa = nc.dram_tensor("wa", [128, IC, D], dt.bfloat16, kind="ExternalInput")
    # ct[p, k, b] = (b_f + hidden @ W_h + b_h)[b, k*128 + p]  (f32)
    ct = nc.dram_tensor("ct", [128, IC, BSH], dt.float32, kind="ExternalInput")
    # ba[p, c] = b_a[c*128 + p]  (f32)
    ba = nc.dram_tensor("ba", [128, DC], dt.float32, kind="ExternalInput")
    # outputs: unnormalized exp, its L-sums, raw weighted sums
    expt = nc.dram_tensor(
        "expt", [BSH, 128, DC, L], dt.bfloat16, kind="ExternalOutput"
    )
    st = nc.dram_tensor("st", [BSH, 128, DC], dt.float32, kind="ExternalOutput")
    zt = nc.dram_tensor("zt", [BSH, 128, DC], dt.float32, kind="ExternalOutput")

    with tile.TileContext(nc) as tc:
        with (
            tc.tile_pool(name="const", bufs=1) as cpool,
            tc.tile_pool(name="xin", bufs=10) as xpool,
            tc.tile_pool(name="t1", bufs=2) as t1pool,
            tc.tile_pool(name="stats", bufs=3) as spool,
            tc.tile_pool(name="scratch", bufs=4) as scpool,
            tc.tile_pool(name="ps1", bufs=4, space="PSUM") as ps1,
            tc.tile_pool(name="ps2", bufs=4, space="PSUM") as ps2,
        ):
            # DMA ring assignment:
            #   sync   : xt loads (pairs 0,2) + h=1 stores
            #   gpsimd : xt loads (pairs 1,3)
            #   scalar : weights + h=0 stores
            def load_pair(pr):
                b0 = 2 * pr
                eng = nc.sync if pr % 2 == 0 else nc.gpsimd
                ng = 4 if pr == 0 else 2          # finer split for pair 0
                gch = DC // ng
                tiles = []
                for g in range(ng):
                    t = xpool.tile(
                        [128, 2, gch, L], dt.bfloat16, tag=f"xtg{gch}",
                        name=f"xt{pr}_{g}",
                    )
                    for h in range(2):
                        eng.dma_start(
                            t[:, h],
                            xt.ap()[b0 + h, :, g * gch : (g + 1) * gch, :],
                        )
                    tiles.append(t)
                return tiles, gch

            xts = {0: load_pair(0)}

            wf_sb = cpool.tile([128, DC, I], dt.bfloat16)
            for kg in range(4):
                nc.scalar.dma_start(
                    wf_sb[:, 4 * kg : 4 * (kg + 1), :],
                    wf.ap()[:, 4 * kg : 4 * (kg + 1), :],
                )
            ct_sb = cpool.tile([128, IC, BSH], dt.float32)
            nc.scalar.dma_start(ct_sb[:], ct.ap())
            ba_sb = cpool.tile([128, DC], dt.float32)
            nc.scalar.dma_start(ba_sb[:], ba.ap())
            wa_sb = cpool.tile([128, IC, D], dt.bfloat16)
            for kg in range(2):
                nc.scalar.dma_start(
                    wa_sb[:, 2 * kg : 2 * (kg + 1), :],
                    wa.ap()[:, 2 * kg : 2 * (kg + 1), :],
                )

            for pr in range(NPAIR):
                b0 = 2 * pr
                xt_sb, gch = xts[pr]

                # ---- matmul 1: t1T[i, (h,l)] = sum_d W_f[d,i] * xT[d,(h,l)]
                # k-outer so each xt chunk is consumed once, as it arrives
                t1s = t1pool.tile([128, IC, 2, L], dt.bfloat16, tag="t1s")
                pt1s = [
                    ps1.tile([128, 2, L], dt.float32, tag="pt1", name=f"pt1_{pr}_{m}")
                    for m in range(IC)
                ]
                for k in range(DC):
                    for m in range(IC):
                        nc.tensor.matmul(
                            pt1s[m][:],
                            wf_sb[:, k, m * 128 : (m + 1) * 128],
                            xt_sb[k // gch][:, :, k % gch, :],
                            start=(k == 0),
                            stop=(k == DC - 1),
                        )
                # bias add (per-partition, per-batch) + cast to bf16 (ACT)
                for m in range(IC):
                    for h in range(2):
                        nc.scalar.activation(
                            t1s[:, m, h, :],
                            pt1s[m][:, h, :],
                            AF.Identity,
                            bias=ct_sb[:, m, b0 + h : b0 + h + 1],
                            scale=1.0,
                        )

                # prefetch next pair's xt (after mm1 so it doesn't gate it)
                if pr + 1 < NPAIR:
                    xts[pr + 1] = load_pair(pr + 1)

                # ---- matmul 2 + exp(+S accum) + z contribution; store ----
                S = spool.tile([128, 2, DC], dt.float32, tag="S")
                zraw = spool.tile([128, 2, DC], dt.float32, tag="zraw")
                e2 = None
                for mc in range(DC):
                    pe = ps2.tile([128, 2, L], dt.float32, tag="pe")
                    for k in range(IC):
                        nc.tensor.matmul(
                            pe[:],
                            wa_sb[:, k, mc * 128 : (mc + 1) * 128],
                            t1s[:, k, :, :],
                            start=(k == 0),
                            stop=(k == IC - 1),
                        )
                    if mc % 2 == 0:
                        e2 = scpool.tile([128, 2, 2, L], dt.bfloat16, tag="e2")
                    # one exp call covers both batches (bias shared)
                    nc.scalar.activation(
                        e2[:, :, mc % 2, :],
                        pe[:],
                        AF.Exp,
                        bias=ba_sb[:, mc : mc + 1],
                        scale=1.0,
                    )
                    for h in range(2):
                        # S = sum_l exp  (cheap DVE copy-with-accum)
                        scs = scpool.tile([128, L], dt.bfloat16, tag=f"scs{h}")
                        nc.vector.tensor_scalar(
                            out=scs[:],
                            in0=e2[:, h, mc % 2, :],
                            scalar1=1.0,
                            scalar2=0.0,
                            op0=ALU.mult,
                            op1=ALU.add,
                            accum_out=S[:, h, mc : mc + 1],
                        )
                        # z contribution from unnormalized exp
                        scr = scpool.tile([128, L], dt.bfloat16, tag=f"scr{h}")
                        nc.vector.scalar_tensor_tensor(
                            out=scr[:],
                            in0=e2[:, h, mc % 2, :],
                            scalar=1.0,
                            in1=xt_sb[mc // gch][:, h, mc % gch, :],
                            op0=ALU.mult,
                            op1=ALU.mult,
                            accum_out=zraw[:, h, mc : mc + 1],
                        )
                    # store exp every 2 chunks (h=0 scalar ring, h=1 sync)
                    if mc % 2 == 1:
                        for h, eng in ((0, nc.scalar), (1, nc.sync)):
                            eng.dma_start(
                                expt.ap()[b0 + h, :, mc - 1 : mc + 1, :],
                                e2[:, h, :, :],
                            )

                for h, eng in ((0, nc.scalar), (1, nc.sync)):
                    eng.dma_start(st.ap()[b0 + h], S[:, h, :])
                    eng.dma_start(zt.ap()[b0 + h], zraw[:, h, :])

    nc.compile()
    return nc


def _get_nc():
    if "nc" not in _CACHE:
        _CACHE["nc"] = _build_nc()
    return _CACHE["nc"]


def _host_prep(fv, hidden, W_f, b_f, W_h, b_h, W_a, b_a):
    import ml_dtypes

    bf16 = ml_dtypes.bfloat16
    c_all = b_f[None, :] + hidden @ W_h + b_h[None, :]          # [B, I] f32

    wf_dev = np.ascontiguousarray(
        W_f.reshape(DC, 128, I).transpose(1, 0, 2)
    ).astype(bf16)                                              # [128, DC, I]
    wa_dev = np.ascontiguousarray(
        W_a.reshape(IC, 128, D).transpose(1, 0, 2)
    ).astype(bf16)                                              # [128, IC, D]
    ba_dev = np.ascontiguousarray(b_a.reshape(DC, 128).T)       # [128, DC]

    # xt[b, p, c, l] = bf16(fv[b, l, c*128+p])
    xt_all = np.ascontiguousarray(
        fv.reshape(B, L, DC, 128).transpose(0, 3, 2, 1)
    ).astype(bf16)                                              # [B, 128, DC, L]

    in_maps = []
    for core in range(NCORES):
        sl = slice(core * BSH, (core + 1) * BSH)
        ct_dev = np.ascontiguousarray(
            c_all[sl].T.reshape(IC, 128, BSH).transpose(1, 0, 2)
        )                                                       # [128, IC, BSH]
        in_maps.append(
            {
                "xt": np.ascontiguousarray(xt_all[sl]),
                "wf": wf_dev,
                "wa": wa_dev,
                "ct": ct_dev,
                "ba": ba_dev,
            }
        )
    return in_maps


def _gather(results):
    """Host-side unshard + softmax normalization (f32)."""
    expt = np.concatenate([np.asarray(r["expt"]) for r in results], axis=0)
    stv = np.concatenate([np.asarray(r["st"]) for r in results], axis=0)
    ztr = np.concatenate([np.asarray(r["zt"]) for r in results], axis=0)

    rec = 1.0 / stv                                             # [B,128,DC] f32
    alpha = np.ascontiguousarray(
        (expt.astype(np.float32) * rec[:, :, :, None]).transpose(0, 3, 2, 1)
    ).reshape(B, L, D)
    z = np.ascontiguousarray((ztr * rec).transpose(0, 2, 1)).reshape(B, D)
    return z, alpha


def kernel(feature_vectors, hidden_state, W_f, b_f, W_h, b_h, W_a, b_a):
    from concourse.bass_utils import run_bass_kernel_spmd

    fv = np.asarray(feature_vectors, dtype=np.float32)
    hidden = np.asarray(hidden_state, dtype=np.float32)
    in_maps = _host_prep(
        fv,
        hidden,
        np.asarray(W_f, dtype=np.float32),
        np.asarray(b_f, dtype=np.float32),
        np.asarray(W_h, dtype=np.float32),
        np.asarray(b_h, dtype=np.float32),
        np.asarray(W_a, dtype=np.float32),
        np.asarray(b_a, dtype=np.float32),
    )

    nc = _get_nc()
    res = run_bass_kernel_spmd(nc, in_maps, core_ids=list(range(NCORES)))
    return _gather(res.results)


# revision 20
# speedup vs baseline: 1.0630x; 1.0630x over previous
"""Trainium2 Bass kernel for nn_Attention_78847009620267.

Reference computation (per batch b):
    t1 = fv[b] @ W_f + (b_f + hidden[b] @ W_h + b_h)     # [L, I]
    e  = t1 @ W_a + b_a                                  # [L, D]
    alpha = softmax(e, axis=L)                           # [L, D]
    z  = sum_l alpha * fv[b]                             # [D]
returns (z [B, D], alpha [B, L, D])

Strategy: data-parallel over batch B=64 across 8 NeuronCores (8 batches per
core), weights replicated.  The whole device kernel runs in the transposed
[D, L] domain so that softmax reductions and bias adds are native
per-partition operations; the host pre-transposes fv into [b, p, c, l] bf16
layout (d = c*128 + p) and post-transposes alpha back.  All matmuls run in
bf16 (fp32 accumulation in PSUM).  The device ships unnormalized exp(e),
its per-(b,d) sums S (free from the ScalarE activation accum_out), and the
raw weighted sums z_raw = sum_l exp*x; the host applies the 1/S
normalization during the gather (exactly the same arithmetic the device
would do, at full f32 precision).

No collectives are needed: each core's outputs depend only on its shard.
"""

import os
import sys

import numpy as np

sys.path.insert(0, "/opt/trn_rl_repo")

B, L, D, I = 64, 196, 2048, 512
NCORES = 8
BSH = B // NCORES          # batches per core = 8
DC = D // 128              # 16 d-chunks
IC = I // 128              # 4 i-chunks
NPAIR = BSH // 2           # process 2 batches per pass (N = 2*L = 392)

_CACHE = {}


def _build_nc():
    """Build the per-core Bass graph (same graph for all 8 cores)."""
    import concourse.bass as bass  # noqa: F401
    import concourse.mybir as mybir
    import concourse.tile as tile
    from concourse import bacc

    dt = mybir.dt
    AF = mybir.ActivationFunctionType
    ALU = mybir.AluOpType

    nc = bacc.Bacc("TRN2", target_bir_lowering=False, debug=False)

    # ---- I/O declarations (per-core shard layouts, see host code below) ----
    # xt[b, p, c, l] = bf16(fv[b, l, c*128 + p])
    xt = nc.dram_tensor("xt", [BSH, 128, DC, L], dt.bfloat16, kind="ExternalInput")
    # wf[p, k, i] = W_f[k*128 + p, i]  (bf16)
    wf = nc.dram_tensor("wf", [128, DC, I], dt.bfloat16, kind="ExternalInput")
    # wa[p, k, d] = W_a[k*128 + p, d]  (bf16)
    wa = nc.dram_tensor("wa", [128, IC, D], dt.bfloat16, kind="ExternalInput")
    # ct[p, k, b] = (b_f + hidden @ W_h + b_h)[b, k*128 + p]  (f32)
    ct = nc.dram_tensor("ct", [128, IC, BSH], dt.float32, kind="ExternalInput")
    # ba[p, c] = b_a[c*128 + p]  (f32)
    ba = nc.dram_tensor("ba", [128, DC], dt.float32, kind="ExternalInput")
    # outputs: unnormalized exp and raw weighted sums (host normalizes)
    expt = nc.dram_tensor(
        "expt", [BSH, 128, DC, L], dt.bfloat16, kind="ExternalOutput"
    )
    zt = nc.dram_tensor("zt", [BSH, 128, DC], dt.float32, kind="ExternalOutput")

    with tile.TileContext(nc) as tc:
        with (
            tc.tile_pool(name="const", bufs=1) as cpool,
            tc.tile_pool(name="xin", bufs=10) as xpool,
            tc.tile_pool(name="t1", bufs=2) as t1pool,
            tc.tile_pool(name="stats", bufs=3) as spool,
            tc.tile_pool(name="scratch", bufs=4) as scpool,
            tc.tile_pool(name="ps1", bufs=4, space="PSUM") as ps1,
            tc.tile_pool(name="ps2", bufs=4, space="PSUM") as ps2,
        ):
            # DMA ring assignment:
            #   sync   : xt loads (pairs 0,2) + h=1 stores
            #   gpsimd : xt loads (pairs 1,3)
            #   scalar : weights + h=0 stores
            def load_pair(pr):
                b0 = 2 * pr
                ng = 4 if pr == 0 else 2          # finer split for pair 0
                gch = DC // ng
                tiles = []
                for g in range(ng):
                    t = xpool.tile(
                        [128, 2, gch, L], dt.bfloat16, tag=f"xtg{gch}",
                        name=f"xt{pr}_{g}",
                    )
                    # batch-halves ride different DMA rings in parallel
                    for h, eng in ((0, nc.sync), (1, nc.gpsimd)):
                        eng.dma_start(
                            t[:, h],
                            xt.ap()[b0 + h, :, g * gch : (g + 1) * gch, :],
                        )
                    tiles.append(t)
                return tiles, gch

            xts = {0: load_pair(0)}

            wf_sb = cpool.tile([128, DC, I], dt.bfloat16)
            for kg in range(4):
                nc.scalar.dma_start(
                    wf_sb[:, 4 * kg : 4 * (kg + 1), :],
                    wf.ap()[:, 4 * kg : 4 * (kg + 1), :],
                )
            ct_sb = cpool.tile([128, IC, BSH], dt.float32)
            nc.scalar.dma_start(ct_sb[:], ct.ap())
            ba_sb = cpool.tile([128, DC], dt.float32)
            nc.scalar.dma_start(ba_sb[:], ba.ap())
            wa_sb = cpool.tile([128, IC, D], dt.bfloat16)
            for kg in range(2):
                nc.scalar.dma_start(
                    wa_sb[:, 2 * kg : 2 * (kg + 1), :],
                    wa.ap()[:, 2 * kg : 2 * (kg + 1), :],
                )

            for pr in range(NPAIR):
                b0 = 2 * pr
                xt_sb, gch = xts[pr]

                # ---- matmul 1: t1T[i, (h,l)] = sum_d W_f[d,i] * xT[d,(h,l)]
                # k-outer so each xt chunk is consumed once, as it arrives
                t1s = t1pool.tile([128, IC, 2, L], dt.bfloat16, tag="t1s")
                pt1s = [
                    ps1.tile([128, 2, L], dt.float32, tag="pt1", name=f"pt1_{pr}_{m}")
                    for m in range(IC)
                ]
                for k in range(DC):
                    for m in range(IC):
                        nc.tensor.matmul(
                            pt1s[m][:],
                            wf_sb[:, k, m * 128 : (m + 1) * 128],
                            xt_sb[k // gch][:, :, k % gch, :],
                            start=(k == 0),
                            stop=(k == DC - 1),
                        )
                # bias add (per-partition, per-batch) + cast to bf16
                # (split across ACT and DVE to halve the burst)
                for m in range(IC):
                    for h in range(2):
                        if m < 2:
                            nc.scalar.activation(
                                t1s[:, m, h, :],
                                pt1s[m][:, h, :],
                                AF.Identity,
                                bias=ct_sb[:, m, b0 + h : b0 + h + 1],
                                scale=1.0,
                            )
                        else:
                            nc.vector.tensor_scalar_add(
                                t1s[:, m, h, :],
                                pt1s[m][:, h, :],
                                ct_sb[:, m, b0 + h : b0 + h + 1],
                            )

                # prefetch next pair's xt (after mm1 so it doesn't gate it)
                if pr + 1 < NPAIR:
                    xts[pr + 1] = load_pair(pr + 1)

                # ---- matmul 2 + exp + z contribution; store ----
                zraw = spool.tile([128, 2, DC], dt.float32, tag="zraw")
                e2 = None
                for mc in range(DC):
                    pe = ps2.tile([128, 2, L], dt.float32, tag="pe")
                    for k in range(IC):
                        nc.tensor.matmul(
                            pe[:],
                            wa_sb[:, k, mc * 128 : (mc + 1) * 128],
                            t1s[:, k, :, :],
                            start=(k == 0),
                            stop=(k == IC - 1),
                        )
                    if mc % 2 == 0:
                        e2 = scpool.tile([128, 2, 2, L], dt.bfloat16, tag="e2")
                    # one exp call covers both batches (bias shared)
                    nc.scalar.activation(
                        e2[:, :, mc % 2, :],
                        pe[:],
                        AF.Exp,
                        bias=ba_sb[:, mc : mc + 1],
                        scale=1.0,
                    )
                    for h in range(2):
                        # z contribution from unnormalized exp
                        scr = scpool.tile([128, L], dt.bfloat16, tag=f"scr{h}")
                        nc.vector.scalar_tensor_tensor(
                            out=scr[:],
                            in0=e2[:, h, mc % 2, :],
                            scalar=1.0,
                            in1=xt_sb[mc // gch][:, h, mc % gch, :],
                            op0=ALU.mult,
                            op1=ALU.mult,
                            accum_out=zraw[:, h, mc : mc + 1],
                        )
                    # store exp every 2 chunks (h=0 scalar ring, h=1 sync)
                    if mc % 2 == 1:
                        for h, eng in ((0, nc.scalar), (1, nc.sync)):
                            eng.dma_start(
                                expt.ap()[b0 + h, :, mc - 1 : mc + 1, :],
                                e2[:, h, :, :],
                            )

                for h, eng in ((0, nc.scalar), (1, nc.sync)):
                    eng.dma_start(zt.ap()[b0 + h], zraw[:, h, :])

    nc.compile()
    return nc


def _get_nc():
    if "nc" not in _CACHE:
        _CACHE["nc"] = _build_nc()
    return _CACHE["nc"]


def _host_prep(fv, hidden, W_f, b_f, W_h, b_h, W_a, b_a):
    import ml_dtypes

    bf16 = ml_dtypes.bfloat16
    c_all = b_f[None, :] + hidden @ W_h + b_h[None, :]          # [B, I] f32

    wf_dev = np.ascontiguousarray(
        W_f.reshape(DC, 128, I).transpose(1, 0, 2)
    ).astype(bf16)                                              # [128, DC, I]
    wa_dev = np.ascontiguousarray(
        W_a.reshape(IC, 128, D).transpose(1, 0, 2)
    ).astype(bf16)                                              # [128, IC, D]
    ba_dev = np.ascontiguousarray(b_a.reshape(DC, 128).T)       # [128, DC]

    # xt[b, p, c, l] = bf16(fv[b, l, c*128+p])
    xt_all = np.ascontiguousarray(
        fv.reshape(B, L, DC, 128).transpose(0, 3, 2, 1)
    ).astype(bf16)                                              # [B, 128, DC, L]

    in_maps = []
    for core in range(NCORES):
        sl = slice(core * BSH, (core + 1) * BSH)
        ct_dev = np.ascontiguousarray(
            c_all[sl].T.reshape(IC, 128, BSH).transpose(1, 0, 2)
        )                                                       # [128, IC, BSH]
        in_maps.append(
            {
                "xt": np.ascontiguousarray(xt_all[sl]),
                "wf": wf_dev,
                "wa": wa_dev,
                "ct": ct_dev,
                "ba": ba_dev,
            }
        )
    return in_maps


def _gather(results):
    """Host-side unshard + softmax normalization (f32)."""
    expt = np.concatenate([np.asarray(r["expt"]) for r in results], axis=0)
    ztr = np.concatenate([np.asarray(r["zt"]) for r in results], axis=0)

    expf = expt.astype(np.float32)
    rec = 1.0 / expf.sum(axis=3)                                # [B,128,DC] f32
    alpha = np.ascontiguousarray(
        (expf * rec[:, :, :, None]).transpose(0, 3, 2, 1)
    ).reshape(B, L, D)
    z = np.ascontiguousarray((ztr * rec).transpose(0, 2, 1)).reshape(B, D)
    return z, alpha


def kernel(feature_vectors, hidden_state, W_f, b_f, W_h, b_h, W_a, b_a):
    from concourse.bass_utils import run_bass_kernel_spmd

    fv = np.asarray(feature_vectors, dtype=np.float32)
    hidden = np.asarray(hidden_state, dtype=np.float32)
    in_maps = _host_prep(
        fv,
        hidden,
        np.asarray(W_f, dtype=np.float32),
        np.asarray(b_f, dtype=np.float32),
        np.asarray(W_h, dtype=np.float32),
        np.asarray(b_h, dtype=np.float32),
        np.asarray(W_a, dtype=np.float32),
        np.asarray(b_a, dtype=np.float32),
    )

    nc = _get_nc()
    res = run_bass_kernel_spmd(nc, in_maps, core_ids=list(range(NCORES)))
    return _gather(res.results)


# revision 39
# speedup vs baseline: 1.1761x; 1.1064x over previous
"""Trainium2 Bass kernel for nn_Attention_78847009620267.

Reference computation (per batch b):
    t1 = fv[b] @ W_f + (b_f + hidden[b] @ W_h + b_h)     # [L, I]
    e  = t1 @ W_a + b_a                                  # [L, D]
    alpha = softmax(e, axis=L)                           # [L, D]
    z  = sum_l alpha * fv[b]                             # [D]
returns (z [B, D], alpha [B, L, D])

Strategy: data-parallel over batch B=64 across 8 NeuronCores (8 batches per
core), weights replicated.  The whole device kernel runs in the transposed
[D, L] domain so that softmax reductions and bias adds are native
per-partition operations; the host pre-transposes fv into [b, p, c, l] bf16
layout (d = c*128 + p) and post-transposes alpha back.  All matmuls run in
bf16 (fp32 accumulation in PSUM).  The device ships unnormalized exp(e),
its per-(b,d) sums S (free from the ScalarE activation accum_out), and the
raw weighted sums z_raw = sum_l exp*x; the host applies the 1/S
normalization during the gather (exactly the same arithmetic the device
would do, at full f32 precision).

No collectives are needed: each core's outputs depend only on its shard.
"""

import os
import sys

import numpy as np

sys.path.insert(0, "/opt/trn_rl_repo")

B, L, D, I = 64, 196, 2048, 512
NCORES = 8
BSH = B // NCORES          # batches per core = 8
DC = D // 128              # 16 d-chunks
IC = I // 128              # 4 i-chunks
NPAIR = BSH // 2           # process 2 batches per pass (N = 2*L = 392)

_CACHE = {}


def _build_nc():
    """Build the per-core Bass graph (same graph for all 8 cores)."""
    import concourse.bass as bass  # noqa: F401
    import concourse.mybir as mybir
    import concourse.tile as tile
    from concourse import bacc

    dt = mybir.dt
    AF = mybir.ActivationFunctionType
    ALU = mybir.AluOpType

    nc = bacc.Bacc("TRN2", target_bir_lowering=False, debug=False)

    # ---- I/O declarations (per-core shard layouts, see host code below) ----
    # xt[b, p, c, l] = bf16(fv[b, l, c*128 + p])
    xt = nc.dram_tensor("xt", [BSH, 128, DC, L], dt.bfloat16, kind="ExternalInput")
    # wf[p, k, i] = W_f[k*128 + p, i]  (bf16)
    wf = nc.dram_tensor("wf", [128, DC, I], dt.bfloat16, kind="ExternalInput")
    # wa[p, k, d] = W_a[k*128 + p, d]  (bf16)
    wa = nc.dram_tensor("wa", [128, IC, D], dt.bfloat16, kind="ExternalInput")
    # ct[p, k, b] = (b_f + hidden @ W_h + b_h)[b, k*128 + p]  (f32)
    ct = nc.dram_tensor("ct", [128, IC, BSH], dt.float32, kind="ExternalInput")
    # ba[p, c] = b_a[c*128 + p]  (f32)
    ba = nc.dram_tensor("ba", [128, DC], dt.float32, kind="ExternalInput")
    # outputs: unnormalized exp and raw weighted sums (host normalizes)
    expt = nc.dram_tensor(
        "expt", [BSH, 128, DC, L], dt.bfloat16, kind="ExternalOutput"
    )
    zt = nc.dram_tensor("zt", [BSH, 128, DC], dt.float32, kind="ExternalOutput")

    with tile.TileContext(nc) as tc:
        with (
            tc.tile_pool(name="const", bufs=1) as cpool,
            tc.tile_pool(name="xin", bufs=4) as xpool,
            tc.tile_pool(name="t1", bufs=2) as t1pool,
            tc.tile_pool(name="stats", bufs=3) as spool,
            tc.tile_pool(name="scratch", bufs=6) as scpool,
            tc.tile_pool(name="ps1", bufs=4, space="PSUM") as ps1,
            tc.tile_pool(name="ps2", bufs=4, space="PSUM") as ps2,
        ):
            # DMA ring assignment:
            #   sync   : xt loads (pairs 0,2) + h=1 stores
            #   gpsimd : xt loads (pairs 1,3)
            #   scalar : weights + h=0 stores
            def load_pair(pr):
                b0 = 2 * pr
                # pair 0: small leading groups so the first matmul starts early
                groups = (2, 2, 4, 8) if pr == 0 else (8, 8)
                tiles = []
                chunk_map = []
                c0 = 0
                for g, gch in enumerate(groups):
                    t = xpool.tile(
                        [128, 2, gch, L], dt.bfloat16, tag=f"xtg{gch}",
                        name=f"xt{pr}_{g}",
                    )
                    # batch-halves ride different DMA rings in parallel;
                    # pair-0's leading groups go on sync for both halves
                    # (the gpsimd/SWDGE Q7 path wakes up late)
                    if pr == 0 and g < 2:
                        engs = ((0, nc.sync), (1, nc.sync))
                    else:
                        engs = ((0, nc.sync), (1, nc.gpsimd))
                    for h, eng in engs:
                        eng.dma_start(
                            t[:, h],
                            xt.ap()[b0 + h, :, c0 : c0 + gch, :],
                        )
                    tiles.append(t)
                    for i in range(gch):
                        chunk_map.append((t, i))
                    c0 += gch
                return chunk_map

            xts = {0: load_pair(0)}

            # wf + bias tables on the scalar ring; wa rides the sync ring
            # (behind pair-0's small h0 loads, ahead of later prefetches)
            # so both weight streams land before they're needed.
            wf_sb = cpool.tile([128, DC, I], dt.bfloat16)
            k0 = 0
            for kg in (1, 1, 2, 4, 4, 4):
                nc.scalar.dma_start(
                    wf_sb[:, k0 : k0 + kg, :],
                    wf.ap()[:, k0 : k0 + kg, :],
                )
                k0 += kg
            ct_sb = cpool.tile([128, IC, BSH], dt.float32)
            nc.scalar.dma_start(ct_sb[:], ct.ap())
            ba_sb = cpool.tile([128, DC], dt.float32)
            nc.scalar.dma_start(ba_sb[:], ba.ap())
            # wa split by d-chunk groups: mm2 consumes one group per ~3us,
            # so the load demand is spread instead of due all-at-once
            wa_sb = cpool.tile([128, IC, D], dt.bfloat16)
            for cg in range(4):
                nc.sync.dma_start(
                    wa_sb[:, :, 512 * cg : 512 * (cg + 1)],
                    wa.ap()[:, :, 512 * cg : 512 * (cg + 1)],
                )

            def mm1_bias(pr, t1s, pt1s):
                """t1s = bf16(pt1s + c): h=0 on ACT, h=1 on DVE."""
                b0 = 2 * pr
                for m in range(IC):
                    nc.scalar.activation(
                        t1s[:, m, 0, :],
                        pt1s[m][:, 0, :],
                        AF.Identity,
                        bias=ct_sb[:, m, b0 : b0 + 1],
                        scale=1.0,
                    )
                    nc.vector.tensor_scalar_add(
                        t1s[:, m, 1, :],
                        pt1s[m][:, 1, :],
                        ct_sb[:, m, b0 + 1 : b0 + 2],
                    )

            def alloc_mm1(pr):
                t1s = t1pool.tile(
                    [128, IC, 2, L], dt.bfloat16, tag="t1s", name=f"t1s_{pr}"
                )
                pt1s = [
                    ps1.tile([128, 2, L], dt.float32, tag="pt1", name=f"pt1_{pr}_{m}")
                    for m in range(IC)
                ]
                return t1s, pt1s

            def mm1_step(pr, pt1s, m, kg):
                """4 mm1 matmuls: bank m, k-group kg (k = 4*kg .. 4*kg+3)."""
                xchunks = xts[pr]
                for k in range(4 * kg, 4 * kg + 4):
                    xtile, xi = xchunks[k]
                    nc.tensor.matmul(
                        pt1s[m][:],
                        wf_sb[:, k, m * 128 : (m + 1) * 128],
                        xtile[:, :, xi, :],
                        start=(k == 0),
                        stop=(k == DC - 1),
                    )

            for pr in range(NPAIR):
                b0 = 2 * pr
                xchunks = xts[pr]

                # ---- matmul 1 (k-outer: consumes xt chunks as they arrive)
                t1s, pt1s = alloc_mm1(pr)
                for kg in range(IC):
                    for m in range(IC):
                        mm1_step(pr, pt1s, m, kg)
                mm1_bias(pr, t1s, pt1s)

                # prefetch next pair's xt (after mm1 so it doesn't gate it)
                if pr + 1 < NPAIR:
                    xts[pr + 1] = load_pair(pr + 1)

                # ---- matmul 2 + exp + z contribution; store ----
                zraw = spool.tile([128, 2, DC], dt.float32, tag="zraw")
                e2 = None
                for mc in range(DC):
                    pe = ps2.tile([128, 2, L], dt.float32, tag="pe")
                    for k in range(IC):
                        nc.tensor.matmul(
                            pe[:],
                            wa_sb[:, k, mc * 128 : (mc + 1) * 128],
                            t1s[:, k, :, :],
                            start=(k == 0),
                            stop=(k == IC - 1),
                        )
                    if mc % 2 == 0:
                        e2 = scpool.tile([128, 2, 2, L], dt.bfloat16, tag="e2")
                    # one exp call covers both batches (bias shared)
                    nc.scalar.activation(
                        e2[:, :, mc % 2, :],
                        pe[:],
                        AF.Exp,
                        bias=ba_sb[:, mc : mc + 1],
                        scale=1.0,
                    )
                    xtile, xi = xchunks[mc]
                    for h in range(2):
                        # z contribution from unnormalized exp
                        scr = scpool.tile([128, L], dt.bfloat16, tag=f"scr{h}")
                        nc.vector.scalar_tensor_tensor(
                            out=scr[:],
                            in0=e2[:, h, mc % 2, :],
                            scalar=1.0,
                            in1=xtile[:, h, xi, :],
                            op0=ALU.mult,
                            op1=ALU.mult,
                            accum_out=zraw[:, h, mc : mc + 1],
                        )
                    # store exp every 2 chunks (h=0 scalar ring, h=1 sync)
                    if mc % 2 == 1:
                        for h, eng in ((0, nc.scalar), (1, nc.sync)):
                            eng.dma_start(
                                expt.ap()[b0 + h, :, mc - 1 : mc + 1, :],
                                e2[:, h, :, :],
                            )

                for h, eng in ((0, nc.scalar), (1, nc.sync)):
                    eng.dma_start(zt.ap()[b0 + h], zraw[:, h, :])

    nc.compile()
    return nc


def _get_nc():
    if "nc" not in _CACHE:
        _CACHE["nc"] = _build_nc()
    return _CACHE["nc"]


def _host_prep(fv, hidden, W_f, b_f, W_h, b_h, W_a, b_a):
    import ml_dtypes

    bf16 = ml_dtypes.bfloat16
    c_all = b_f[None, :] + hidden @ W_h + b_h[None, :]          # [B, I] f32

    wf_dev = np.ascontiguousarray(
        W_f.reshape(DC, 128, I).transpose(1, 0, 2)
    ).astype(bf16)                                              # [128, DC, I]
    wa_dev = np.ascontiguousarray(
        W_a.reshape(IC, 128, D).transpose(1, 0, 2)
    ).astype(bf16)                                              # [128, IC, D]
    ba_dev = np.ascontiguousarray(b_a.reshape(DC, 128).T)       # [128, DC]

    # xt[b, p, c, l] = bf16(fv[b, l, c*128+p])
    xt_all = np.ascontiguousarray(
        fv.reshape(B, L, DC, 128).transpose(0, 3, 2, 1)
    ).astype(bf16)                                              # [B, 128, DC, L]

    in_maps = []
    for core in range(NCORES):
        sl = slice(core * BSH, (core + 1) * BSH)
        ct_dev = np.ascontiguousarray(
            c_all[sl].T.reshape(IC, 128, BSH).transpose(1, 0, 2)
        )                                                       # [128, IC, BSH]
        in_maps.append(
            {
                "xt": np.ascontiguousarray(xt_all[sl]),
                "wf": wf_dev,
                "wa": wa_dev,
                "ct": ct_dev,
                "ba": ba_dev,
            }
        )
    return in_maps


def _gather(results):
    """Host-side unshard + softmax normalization (f32)."""
    expt = np.concatenate([np.asarray(r["expt"]) for r in results], axis=0)
    ztr = np.concatenate([np.asarray(r["zt"]) for r in results], axis=0)

    expf = expt.astype(np.float32)
    rec = 1.0 / expf.sum(axis=3)                                # [B,128,DC] f32
    alpha = np.ascontiguousarray(
        (expf * rec[:, :, :, None]).transpose(0, 3, 2, 1)
    ).reshape(B, L, D)
    z = np.ascontiguousarray((ztr * rec).transpose(0, 2, 1)).reshape(B, D)
    return z, alpha


def kernel(feature_vectors, hidden_state, W_f, b_f, W_h, b_h, W_a, b_a):
    from concourse.bass_utils import run_bass_kernel_spmd

    fv = np.asarray(feature_vectors, dtype=np.float32)
    hidden = np.asarray(hidden_state, dtype=np.float32)
    in_maps = _host_prep(
        fv,
        hidden,
        np.asarray(W_f, dtype=np.float32),
        np.asarray(b_f, dtype=np.float32),
        np.asarray(W_h, dtype=np.float32),
        np.asarray(b_h, dtype=np.float32),
        np.asarray(W_a, dtype=np.float32),
        np.asarray(b_a, dtype=np.float32),
    )

    nc = _get_nc()
    last_err = None
    for attempt in range(3):
        try:
            res = run_bass_kernel_spmd(nc, in_maps, core_ids=list(range(NCORES)))
            return _gather(res.results)
        except Exception as e:  # transient NRT/device errors: retry
            last_err = e
            import time

            time.sleep(2.0)
    raise last_err


# revision 42
# speedup vs baseline: 1.1766x; 1.0004x over previous
"""Trainium2 Bass kernel for nn_Attention_78847009620267.

Reference computation (per batch b):
    t1 = fv[b] @ W_f + (b_f + hidden[b] @ W_h + b_h)     # [L, I]
    e  = t1 @ W_a + b_a                                  # [L, D]
    alpha = softmax(e, axis=L)                           # [L, D]
    z  = sum_l alpha * fv[b]                             # [D]
returns (z [B, D], alpha [B, L, D])

Strategy: data-parallel over batch B=64 across 8 NeuronCores (8 batches per
core), weights replicated.  The whole device kernel runs in the transposed
[D, L] domain so that softmax reductions and bias adds are native
per-partition operations; the host pre-transposes fv into [b, p, c, l] bf16
layout (d = c*128 + p) and post-transposes alpha back.  All matmuls run in
bf16 (fp32 accumulation in PSUM).  The device ships unnormalized exp(e),
its per-(b,d) sums S (free from the ScalarE activation accum_out), and the
raw weighted sums z_raw = sum_l exp*x; the host applies the 1/S
normalization during the gather (exactly the same arithmetic the device
would do, at full f32 precision).

No collectives are needed: each core's outputs depend only on its shard.
"""

import os
import sys

import numpy as np

sys.path.insert(0, "/opt/trn_rl_repo")

B, L, D, I = 64, 196, 2048, 512
NCORES = 8
BSH = B // NCORES          # batches per core = 8
DC = D // 128              # 16 d-chunks
IC = I // 128              # 4 i-chunks
NPAIR = BSH // 2           # process 2 batches per pass (N = 2*L = 392)

_CACHE = {}


def _build_nc():
    """Build the per-core Bass graph (same graph for all 8 cores)."""
    import concourse.bass as bass  # noqa: F401
    import concourse.mybir as mybir
    import concourse.tile as tile
    from concourse import bacc

    dt = mybir.dt
    AF = mybir.ActivationFunctionType
    ALU = mybir.AluOpType

    nc = bacc.Bacc("TRN2", target_bir_lowering=False, debug=False)

    # ---- I/O declarations (per-core shard layouts, see host code below) ----
    # xt[b, p, c, l] = bf16(fv[b, l, c*128 + p])
    xt = nc.dram_tensor("xt", [BSH, 128, DC, L], dt.bfloat16, kind="ExternalInput")
    # wf[p, k, i] = W_f[k*128 + p, i]  (bf16)
    wf = nc.dram_tensor("wf", [128, DC, I], dt.bfloat16, kind="ExternalInput")
    # wa[p, k, d] = W_a[k*128 + p, d]  (bf16)
    wa = nc.dram_tensor("wa", [128, IC, D], dt.bfloat16, kind="ExternalInput")
    # ct[p, k, b] = (b_f + hidden @ W_h + b_h)[b, k*128 + p]  (f32)
    ct = nc.dram_tensor("ct", [128, IC, BSH], dt.float32, kind="ExternalInput")
    # ba[p, c] = b_a[c*128 + p]  (f32)
    ba = nc.dram_tensor("ba", [128, DC], dt.float32, kind="ExternalInput")
    # outputs: unnormalized exp and raw weighted sums (host normalizes)
    expt = nc.dram_tensor(
        "expt", [BSH, 128, DC, L], dt.bfloat16, kind="ExternalOutput"
    )
    zt = nc.dram_tensor("zt", [BSH, 128, DC], dt.float32, kind="ExternalOutput")

    with tile.TileContext(nc) as tc:
        with (
            tc.tile_pool(name="const", bufs=1) as cpool,
            tc.tile_pool(name="xin", bufs=4) as xpool,
            tc.tile_pool(name="t1", bufs=2) as t1pool,
            tc.tile_pool(name="stats", bufs=3) as spool,
            tc.tile_pool(name="scratch", bufs=8) as scpool,
            tc.tile_pool(name="ps1", bufs=4, space="PSUM") as ps1,
            tc.tile_pool(name="ps2", bufs=4, space="PSUM") as ps2,
        ):
            # DMA ring assignment:
            #   sync   : xt loads (pairs 0,2) + h=1 stores
            #   gpsimd : xt loads (pairs 1,3)
            #   scalar : weights + h=0 stores
            def load_pair(pr):
                b0 = 2 * pr
                # pair 0: small leading groups so the first matmul starts early
                groups = (2, 2, 4, 8) if pr == 0 else (8, 8)
                tiles = []
                chunk_map = []
                c0 = 0
                for g, gch in enumerate(groups):
                    t = xpool.tile(
                        [128, 2, gch, L], dt.bfloat16, tag=f"xtg{gch}",
                        name=f"xt{pr}_{g}",
                    )
                    # batch-halves ride different DMA rings in parallel;
                    # pair-0's leading groups go on sync for both halves
                    # (the gpsimd/SWDGE Q7 path wakes up late)
                    if pr == 0 and g < 2:
                        engs = ((0, nc.sync), (1, nc.sync))
                    else:
                        engs = ((0, nc.sync), (1, nc.gpsimd))
                    for h, eng in engs:
                        eng.dma_start(
                            t[:, h],
                            xt.ap()[b0 + h, :, c0 : c0 + gch, :],
                        )
                    tiles.append(t)
                    for i in range(gch):
                        chunk_map.append((t, i))
                    c0 += gch
                return chunk_map

            xts = {0: load_pair(0)}

            # wf + bias tables on the scalar ring; wa rides the sync ring
            # (behind pair-0's small h0 loads, ahead of later prefetches)
            # so both weight streams land before they're needed.
            wf_sb = cpool.tile([128, DC, I], dt.bfloat16)
            k0 = 0
            for kg in (1, 1, 2, 4, 4, 4):
                nc.scalar.dma_start(
                    wf_sb[:, k0 : k0 + kg, :],
                    wf.ap()[:, k0 : k0 + kg, :],
                )
                k0 += kg
            ct_sb = cpool.tile([128, IC, BSH], dt.float32)
            nc.scalar.dma_start(ct_sb[:], ct.ap())
            ba_sb = cpool.tile([128, DC], dt.float32)
            nc.scalar.dma_start(ba_sb[:], ba.ap())
            # wa split by d-chunk groups: mm2 consumes one group per ~3us,
            # so the load demand is spread instead of due all-at-once
            wa_sb = cpool.tile([128, IC, D], dt.bfloat16)
            for cg in range(4):
                nc.sync.dma_start(
                    wa_sb[:, :, 512 * cg : 512 * (cg + 1)],
                    wa.ap()[:, :, 512 * cg : 512 * (cg + 1)],
                )

            def mm1_bias(pr, t1s, pt1s):
                """t1s = bf16(pt1s + c): h=0 on ACT, h=1 on DVE."""
                b0 = 2 * pr
                for m in range(IC):
                    nc.scalar.activation(
                        t1s[:, m, 0, :],
                        pt1s[m][:, 0, :],
                        AF.Identity,
                        bias=ct_sb[:, m, b0 : b0 + 1],
                        scale=1.0,
                    )
                    nc.vector.tensor_scalar_add(
                        t1s[:, m, 1, :],
                        pt1s[m][:, 1, :],
                        ct_sb[:, m, b0 + 1 : b0 + 2],
                    )

            def alloc_mm1(pr):
                t1s = t1pool.tile(
                    [128, IC, 2, L], dt.bfloat16, tag="t1s", name=f"t1s_{pr}"
                )
                pt1s = [
                    ps1.tile([128, 2, L], dt.float32, tag="pt1", name=f"pt1_{pr}_{m}")
                    for m in range(IC)
                ]
                return t1s, pt1s

            def mm1_step(pr, pt1s, m, kg):
                """4 mm1 matmuls: bank m, k-group kg (k = 4*kg .. 4*kg+3)."""
                xchunks = xts[pr]
                for k in range(4 * kg, 4 * kg + 4):
                    xtile, xi = xchunks[k]
                    nc.tensor.matmul(
                        pt1s[m][:],
                        wf_sb[:, k, m * 128 : (m + 1) * 128],
                        xtile[:, :, xi, :],
                        start=(k == 0),
                        stop=(k == DC - 1),
                    )

            for pr in range(NPAIR):
                b0 = 2 * pr
                xchunks = xts[pr]

                # ---- matmul 1 (k-outer: consumes xt chunks as they arrive)
                t1s, pt1s = alloc_mm1(pr)
                for kg in range(IC):
                    for m in range(IC):
                        mm1_step(pr, pt1s, m, kg)
                mm1_bias(pr, t1s, pt1s)

                # prefetch next pair's xt (after mm1 so it doesn't gate it)
                if pr + 1 < NPAIR:
                    xts[pr + 1] = load_pair(pr + 1)

                # ---- matmul 2 + exp + z contribution; store ----
                zraw = spool.tile([128, 2, DC], dt.float32, tag="zraw")
                e2 = None
                for mc in range(DC):
                    pe = ps2.tile([128, 2, L], dt.float32, tag="pe")
                    for k in range(IC):
                        nc.tensor.matmul(
                            pe[:],
                            wa_sb[:, k, mc * 128 : (mc + 1) * 128],
                            t1s[:, k, :, :],
                            start=(k == 0),
                            stop=(k == IC - 1),
                        )
                    if mc % 2 == 0:
                        e2 = scpool.tile([128, 2, 2, L], dt.bfloat16, tag="e2")
                    # one exp call covers both batches (bias shared)
                    nc.scalar.activation(
                        e2[:, :, mc % 2, :],
                        pe[:],
                        AF.Exp,
                        bias=ba_sb[:, mc : mc + 1],
                        scale=1.0,
                    )
                    xtile, xi = xchunks[mc]
                    for h in range(2):
                        # z contribution from unnormalized exp
                        scr = scpool.tile([128, L], dt.bfloat16, tag=f"scr{h}")
                        nc.vector.scalar_tensor_tensor(
                            out=scr[:],
                            in0=e2[:, h, mc % 2, :],
                            scalar=1.0,
                            in1=xtile[:, h, xi, :],
                            op0=ALU.mult,
                            op1=ALU.mult,
                            accum_out=zraw[:, h, mc : mc + 1],
                        )
                    # store exp every 2 chunks (h=0 scalar ring, h=1 sync)
                    if mc % 2 == 1:
                        for h, eng in ((0, nc.scalar), (1, nc.sync)):
                            eng.dma_start(
                                expt.ap()[b0 + h, :, mc - 1 : mc + 1, :],
                                e2[:, h, :, :],
                            )

                for h, eng in ((0, nc.scalar), (1, nc.sync)):
                    eng.dma_start(zt.ap()[b0 + h], zraw[:, h, :])

    nc.compile()
    return nc


def _get_nc():
    if "nc" not in _CACHE:
        _CACHE["nc"] = _build_nc()
    return _CACHE["nc"]


def _host_prep(fv, hidden, W_f, b_f, W_h, b_h, W_a, b_a):
    import ml_dtypes

    bf16 = ml_dtypes.bfloat16
    c_all = b_f[None, :] + hidden @ W_h + b_h[None, :]          # [B, I] f32

    wf_dev = np.ascontiguousarray(
        W_f.reshape(DC, 128, I).transpose(1, 0, 2)
    ).astype(bf16)                                              # [128, DC, I]
    wa_dev = np.ascontiguousarray(
        W_a.reshape(IC, 128, D).transpose(1, 0, 2)
    ).astype(bf16)                                              # [128, IC, D]
    ba_dev = np.ascontiguousarray(b_a.reshape(DC, 128).T)       # [128, DC]

    # xt[b, p, c, l] = bf16(fv[b, l, c*128+p])
    xt_all = np.ascontiguousarray(
        fv.reshape(B, L, DC, 128).transpose(0, 3, 2, 1)
    ).astype(bf16)                                              # [B, 128, DC, L]

    in_maps = []
    for core in range(NCORES):
        sl = slice(core * BSH, (core + 1) * BSH)
        ct_dev = np.ascontiguousarray(
            c_all[sl].T.reshape(IC, 128, BSH).transpose(1, 0, 2)
        )                                                       # [128, IC, BSH]
        in_maps.append(
            {
                "xt": np.ascontiguousarray(xt_all[sl]),
                "wf": wf_dev,
                "wa": wa_dev,
                "ct": ct_dev,
                "ba": ba_dev,
            }
        )
    return in_maps


def _gather(results):
    """Host-side unshard + softmax normalization (f32)."""
    expt = np.concatenate([np.asarray(r["expt"]) for r in results], axis=0)
    ztr = np.concatenate([np.asarray(r["zt"]) for r in results], axis=0)

    expf = expt.astype(np.float32)
    rec = 1.0 / expf.sum(axis=3)                                # [B,128,DC] f32
    alpha = np.ascontiguousarray(
        (expf * rec[:, :, :, None]).transpose(0, 3, 2, 1)
    ).reshape(B, L, D)
    z = np.ascontiguousarray((ztr * rec).transpose(0, 2, 1)).reshape(B, D)
    return z, alpha


def kernel(feature_vectors, hidden_state, W_f, b_f, W_h, b_h, W_a, b_a):
    from concourse.bass_utils import run_bass_kernel_spmd

    fv = np.asarray(feature_vectors, dtype=np.float32)
    hidden = np.asarray(hidden_state, dtype=np.float32)
    in_maps = _host_prep(
        fv,
        hidden,
        np.asarray(W_f, dtype=np.float32),
        np.asarray(b_f, dtype=np.float32),
        np.asarray(W_h, dtype=np.float32),
        np.asarray(b_h, dtype=np.float32),
        np.asarray(W_a, dtype=np.float32),
        np.asarray(b_a, dtype=np.float32),
    )

    nc = _get_nc()
    last_err = None
    for attempt in range(3):
        try:
            res = run_bass_kernel_spmd(nc, in_maps, core_ids=list(range(NCORES)))
            return _gather(res.results)
        except Exception as e:  # transient NRT/device errors: retry
            last_err = e
            import time

            time.sleep(2.0)
    raise last_err
